# revision 22
# baseline (speedup 1.0000x reference)
"""Multi-Head Latent Attention (MLA) Trainium2 kernel, v6 (fp16).

Problem (hardcoded): B=2, S=2048, D_MODEL=2048, H=16, HEAD_DIM=128,
D_LATENT=512 (D_QK=256 / D_V=256), ROPE_DIM=64, fp32 in/out.

Reference semantics: q = concat([q_no_rope(1024), q_rope(1024)]).reshape(16
heads x 128), so heads 0-7 take both 64-dim halves from the latent
decompression and heads 8-15 take both halves from the rope projection of x;
RoPE rotates dims 64:128 of every head.

Sharding: 8 cores = 2 batches x 4 head-groups; core (b, hg) owns heads
[2hg, 2hg+1, 8+2hg, 8+2hg+1] (2 decompression + 2 rope-projection heads),
computes the shared latent for its batch redundantly, and produces a partial
output projection (its heads' rows of W_out), transposed [e, q]. The host
sums the 4 partials per batch.

v3 structure:
  - fp16 operands everywhere (host converts), f32 psum accumulation
  - stage1 n-outer: xT streamed in 512-column chunks so the PE starts after
    ~6MB of DMA instead of 13MB; 8 psum banks = 8 m-blocks per chunk
  - attention per (qc, h): scores in [128,1024] kc-pair psums, exp with
    bias=-3 shift on scalar, av accumulating in one psum bank
  - softmax denominator: exp-tile accumulation split between DVE and GpSimd
    (both SBUF-only), folded then column-summed with one small ones-matmul
  - out-projection matmuls of q-chunk qc-1 are interleaved two-per-pair-step
    into attention(qc) so the PE fills the exp-latency gaps
"""

import math

import numpy as np

B = 2
S = 2048
D = 2048
H4 = 4            # heads per core
HD = 128          # head dim
DL = 512          # d_latent
DQK = 256
RD = 64           # rope dim
NC = 8            # cores

NQ = S // 512     # 4 q chunks of 512
NK = S // 128     # 16 k chunks of 128
NP = NK // 2      # 8 kc pairs
KD = D // 128     # 16 contraction chunks for stage 1

SCALE = 1.0 / math.sqrt(HD)
EXP_SHIFT = -3.0  # exp(s*SCALE - 3): cancels in softmax, keeps fp16 range

_prog_cache = {}


def _build_program():
    import contextlib

    import concourse.tile as tile
    from concourse import bacc, mybir

    f16 = mybir.dt.float16
    f32 = mybir.dt.float32
    EXP = mybir.ActivationFunctionType.Exp
    IDENT = mybir.ActivationFunctionType.Identity

    nc = bacc.Bacc("TRN2", target_bir_lowering=False, debug=False,
                   num_devices=1)

    xT = nc.dram_tensor("xT", [D, S], f16, kind="ExternalInput")
    w_big = nc.dram_tensor("w_big", [D, 1024], f16, kind="ExternalInput")
    w_qk = nc.dram_tensor("w_qk", [DQK, 512], f16, kind="ExternalInput")
    w_v = nc.dram_tensor("w_v", [DQK, 512], f16, kind="ExternalInput")
    w_o = nc.dram_tensor("w_o", [DL, D], f16, kind="ExternalInput")
    cos4_d = nc.dram_tensor("cos4", [128, S], f16, kind="ExternalInput")
    sin4s_d = nc.dram_tensor("sin4s", [64, S], f16, kind="ExternalInput")
    out_d = nc.dram_tensor("out", [D, S], f16, kind="ExternalOutput")

    with tile.TileContext(nc, pool_alloc_mode="queue") as tc:
        with contextlib.ExitStack() as ctx:
            # ---------------- persistent pools ----------------
            ones_p = ctx.enter_context(tc.tile_pool(name="onesp", bufs=1))
            qk_p = ctx.enter_context(tc.tile_pool(name="qk", bufs=1))
            v_p = ctx.enter_context(tc.tile_pool(name="vp", bufs=1))
            consts_p = ctx.enter_context(tc.tile_pool(name="consts", bufs=1))
            wdec_p = ctx.enter_context(tc.tile_pool(name="wdec", bufs=1))

            ones_f32 = ones_p.tile([128, 128], f32)
            nc.gpsimd.memset(ones_f32[:], 1.0)
            ones16 = ones_p.tile([128, 128], f16)
            nc.vector.tensor_copy(ones16[:], ones_f32[:])
            warm = ones_p.tile([128, 1], f32)
            nc.scalar.activation(warm[:], ones_f32[:, 0:1], EXP)
            shift = ones_p.tile([128, 1], f32)
            nc.gpsimd.memset(shift[:], EXP_SHIFT)

            # qkT[0..3] = q heads 0..3, qkT[4..7] = k heads 0..3
            # (local heads 0,1 = dec heads; 2,3 = rope-proj heads)
            qkT = [qk_p.tile([128, S], f16, name=f"qkT{i}", tag=f"qk{i}")
                   for i in range(8)]
            # v pair tiles: v_pair[p] = v_nat[2p] | v_nat[2p+1]
            v_pair = [v_p.tile([128, 1024], f16, name=f"v{p}", tag=f"v{p}")
                      for p in range(NP)]

            # ---------------- phase A: stage1 + dec + v + rope ----------
            # latn[i] = latent rows [128i,128i+128) x [S], fp16
            # (i=0,1: c_qk; i=2,3: c_v)
            lat_cm = tc.tile_pool(name="lat", bufs=1)
            lat_p = lat_cm.__enter__()
            latn = [lat_p.tile([128, S], f16, name=f"latT{i}", tag=f"lat{i}")
                    for i in range(4)]

            wbig_cm = tc.tile_pool(name="wbig", bufs=1)
            wbig_p = wbig_cm.__enter__()
            # xt ring: 32 x [128,512] column-chunk tiles (2 n-chunks in
            # flight); DMA of chunk n+2 waits for chunk n's slots to free
            xt_cm = tc.tile_pool(name="xt", bufs=32)
            xt_p = xt_cm.__enter__()

            wbig_sb = []
            xt_sb = [[None] * KD for _ in range(4)]
            for k in range(KD):
                w_t = wbig_p.tile([128, 1024], f16, name=f"wb{k}")
                nc.sync.dma_start(
                    w_t[:], w_big.ap()[k * 128:(k + 1) * 128, :])
                wbig_sb.append(w_t)
                x_t = xt_p.tile([128, 512], f16, name=f"xt0_{k}", tag="xt")
                nc.sync.dma_start(x_t[:],
                                  xT.ap()[k * 128:(k + 1) * 128, 0:512])
                xt_sb[0][k] = x_t
            for n in range(1, 4):
                for k in range(KD):
                    x_t = xt_p.tile([128, 512], f16, name=f"xt{n}_{k}",
                                    tag="xt")
                    nc.sync.dma_start(
                        x_t[:],
                        xT.ap()[k * 128:(k + 1) * 128,
                                n * 512:(n + 1) * 512])
                    xt_sb[n][k] = x_t

            cos4 = consts_p.tile([128, S], f16)
            nc.sync.dma_start(cos4[:], cos4_d.ap()[:])
            sin4s = consts_p.tile([64, S], f16)
            nc.sync.dma_start(sin4s[:], sin4s_d.ap()[:])

            wqk_sb = []
            for l in range(2):
                w_t = wdec_p.tile([128, 512], f16, name=f"wqk{l}")
                nc.sync.dma_start(w_t[:], w_qk.ap()[l * 128:(l + 1) * 128, :])
                wqk_sb.append(w_t)
            wv_sb = []
            for l in range(2):
                w_t = wdec_p.tile([128, 512], f16, name=f"wv{l}")
                nc.sync.dma_start(w_t[:], w_v.ap()[l * 128:(l + 1) * 128, :])
                wv_sb.append(w_t)

            psA_cm = tc.tile_pool(name="psA", bufs=8, space="PSUM")
            psA_p = psA_cm.__enter__()

            sw_cm = tc.tile_pool(name="swp", bufs=2)
            sw_p = sw_cm.__enter__()
            scr_cm = tc.tile_pool(name="scr", bufs=2)
            scr_p = scr_cm.__enter__()

            def rope_tile(i):
                # in-place RoPE on rows 64:128 of qkT[i]
                t = qkT[i]
                sw = sw_p.tile([64, S], f16, name=f"sw{i}", tag="sw")
                nc.sync.dma_start(sw[0:32, :], t[96:128, :])
                nc.sync.dma_start(sw[32:64, :], t[64:96, :])
                tmp_sin = scr_p.tile([64, S], f16, name="tsin", tag="scr")
                nc.vector.tensor_mul(tmp_sin[:], sw[:], sin4s[:])
                tmp_cos = scr_p.tile([64, S], f16, name="tcos", tag="scr")
                nc.vector.tensor_mul(tmp_cos[:], t[64:128, :],
                                     cos4[64:128, :])
                nc.vector.tensor_add(t[64:128, :], tmp_cos[:], tmp_sin[:])

            # stage1: n-outer, all 8 m-blocks per 512-column chunk
            s1_dst = [latn[0], latn[1], latn[2], latn[3],
                      qkT[2], qkT[3], qkT[6], qkT[7]]
            for n in range(4):
                nsl = slice(n * 512, (n + 1) * 512)
                M_ORD = (4, 6, 5, 7, 0, 1, 2, 3)  # rope-head blocks first
                pss = {m: psA_p.tile([128, 512], f32, name=f"ps_{n}_{m}",
                                     tag="psA") for m in M_ORD}
                for k in range(KD):
                    for m in M_ORD:
                        nc.tensor.matmul(
                            pss[m][:],
                            wbig_sb[k][:, m * 128:(m + 1) * 128],
                            xt_sb[n][k][:],
                            start=(k == 0), stop=(k == KD - 1),
                        )
                for m in M_ORD:
                    nc.scalar.activation(s1_dst[m][:, nsl], pss[m][:], IDENT)

            # rope-proj heads rotate first (feed attention heads 2,3)
            rope_tile(2)
            rope_tile(6)

            def dec_mt(mt, on_dve=False):
                # q/k decompression for dec head mt -> qkT[[0,1,4,5][mt]]
                dst = qkT[[0, 1, 4, 5][mt]]
                pss = [psA_p.tile([128, 512], f32, name=f"psd{mt}_{n2}",
                                  tag="psA") for n2 in range(4)]
                for l in range(2):
                    for n2 in range(4):
                        nc.tensor.matmul(
                            pss[n2][:],
                            wqk_sb[l][:, mt * 128:(mt + 1) * 128],
                            latn[l][:, n2 * 512:(n2 + 1) * 512],
                            start=(l == 0), stop=(l == 1),
                        )
                for n2 in range(4):
                    if on_dve:
                        nc.vector.tensor_copy(
                            dst[:, n2 * 512:(n2 + 1) * 512], pss[n2][:])
                    else:
                        nc.scalar.activation(
                            dst[:, n2 * 512:(n2 + 1) * 512], pss[n2][:],
                            IDENT)

            # v decompression first (feeds av of heads 2,3; casts on DVE
            # so the scalar queue reaches the first exps quickly)
            for sc in range(NK):
                ps = psA_p.tile([128, 512], f32, name=f"psv{sc}", tag="psA")
                for l in range(2):
                    nc.tensor.matmul(
                        ps[:],
                        latn[2 + l][:, sc * 128:(sc + 1) * 128],
                        wv_sb[l][:],
                        start=(l == 0), stop=(l == 1),
                    )
                nc.scalar.activation(
                    v_pair[sc // 2][:, (sc % 2) * 512:(sc % 2 + 1) * 512],
                    ps[:], IDENT)
            dec_mt(0)
            dec_mt(2)
            dec_mt(1)
            dec_mt(3)
            rope_tile(3)
            rope_tile(7)
            rope_tile(0)
            rope_tile(4)
            rope_tile(1)
            rope_tile(5)

            scr_cm.__exit__(None, None, None)
            sw_cm.__exit__(None, None, None)
            psA_cm.__exit__(None, None, None)
            xt_cm.__exit__(None, None, None)
            wbig_cm.__exit__(None, None, None)
            lat_cm.__exit__(None, None, None)

            # ---------------- phase B: attention + out projection --------
            # PSUM: ps_s 2x[128,1024] (4 banks) + ps_c 2x[128,512] (2) +
            #       ps_o 1x[128,1024] (2) = 8 banks
            with tc.tile_pool(name="wo", bufs=1) as wo_p, \
                 tc.tile_pool(name="exp", bufs=3) as exp_p, \
                 tc.tile_pool(name="den", bufs=4) as den_p, \
                 tc.tile_pool(name="rden", bufs=2) as rden_p, \
                 tc.tile_pool(name="ctx", bufs=8) as ctx_p, \
                 tc.tile_pool(name="stage", bufs=3) as stage_p, \
                 tc.tile_pool(name="ps_s", bufs=2, space="PSUM") as ps_s_p, \
                 tc.tile_pool(name="ps_c", bufs=2, space="PSUM") as ps_c_p, \
                 tc.tile_pool(name="ps_o", bufs=1, space="PSUM") as ps_o_p:
                wo_sb = []
                for kk in range(4):
                    w_t = wo_p.tile([128, D], f16, name=f"wo{kk}")
                    nc.sync.dma_start(
                        w_t[:], w_o.ap()[kk * 128:(kk + 1) * 128, :])
                    wo_sb.append(w_t)

                def make_out_emitters(qc, ctx_tiles, final=False):
                    """64 single-matmul emitters for out-proj of q-chunk qc,
                    with psum alloc / cast / dma folded into the stream."""
                    qsl = slice(qc * 512, (qc + 1) * 512)
                    state = {}

                    def emit_one(i):
                        mp, r = divmod(i, 8)
                        half, kk = divmod(r, 4)
                        m = 2 * mp + half
                        if r == 0:
                            use_s = final and (mp % 3 != 0)
                            pool = ps_s_p if use_s else ps_o_p
                            tag2 = "pss" if use_s else "pso"
                            state["ps"] = pool.tile(
                                [128, 1024], f32, name="pso", tag=tag2)
                        nc.tensor.matmul(
                            state["ps"][:, half * 512:(half + 1) * 512],
                            wo_sb[kk][:, m * 128:(m + 1) * 128],
                            ctx_tiles[kk][:],
                            start=(kk == 0), stop=(kk == 3),
                        )
                        if r in (3, 7):
                            hf = r // 4
                            st = stage_p.tile([128, 512], f16, name="stg",
                                              tag="stage")
                            psl = state["ps"][:, hf * 512:(hf + 1) * 512]
                            if hf == 0:
                                nc.vector.tensor_copy(st[:], psl)
                            else:
                                nc.scalar.activation(st[:], psl, IDENT)
                            nc.sync.dma_start(
                                out_d.ap()[m * 128:(m + 1) * 128, qsl],
                                st[:])

                    return [lambda i=i: emit_one(i) for i in range(64)]

                prev_out = []
                pending_fin = [None]
                for qc in range(NQ):
                    qsl = slice(qc * 512, (qc + 1) * 512)
                    out_q = list(prev_out)  # out-proj work of qc-1
                    oi = 0

                    def pop_out(k2, out_q=out_q):
                        nonlocal oi
                        for _ in range(k2):
                            if oi < len(out_q):
                                out_q[oi]()
                                oi += 1

                    ctx_sb = [None] * H4
                    for h in (2, 3, 0, 1):
                        ps_ctx = ps_c_p.tile([128, 512], f32, name="psc",
                                             tag="psc")
                        acc_d = den_p.tile([128, 1024], f16, name="accd",
                                           tag="acc")
                        exps = []

                        def emit_scores(p, h=h, qsl=qsl, exps=exps):
                            ps = ps_s_p.tile([128, 1024], f32, name="pss",
                                             tag="pss")
                            for j in range(2):
                                kc = 2 * p + j
                                nc.tensor.matmul(
                                    ps[:, j * 512:(j + 1) * 512],
                                    qkT[4 + h][:, kc * 128:(kc + 1) * 128],
                                    qkT[h][:, qsl],
                                    start=True, stop=True,
                                )
                            e = exp_p.tile([128, 1024], f16, name="expT",
                                           tag="exp")
                            nc.scalar.activation(e[:], ps[:], EXP,
                                                 bias=shift[:], scale=SCALE)
                            exps.append(e)

                        def emit_av(p, h=h, exps=exps, ps_ctx=ps_ctx,
                                    acc_d=acc_d):
                            e = exps[p]
                            for j in range(2):
                                kc = 2 * p + j
                                nc.tensor.matmul(
                                    ps_ctx[:],
                                    v_pair[p][:, j * 512 + h * 128:
                                              j * 512 + (h + 1) * 128],
                                    e[:, j * 512:(j + 1) * 512],
                                    start=(kc == 0), stop=(kc == NK - 1),
                                )
                            # denominator accumulation on DVE (fp16 2x)
                            if p == 0:
                                nc.vector.tensor_copy(acc_d[:], e[:])
                            else:
                                nc.vector.tensor_add(acc_d[:], acc_d[:],
                                                     e[:])

                        emit_scores(0)
                        emit_scores(1)
                        pop_out(2)
                        emit_av(0)
                        for p in range(2, NP):
                            emit_scores(p)
                            pop_out(2)
                            emit_av(p - 1)
                            if p == 3 and pending_fin[0] is not None:
                                # previous head's den/ctx finisher, late
                                # enough that its DVE fold input is ready
                                pending_fin[0]()
                                pending_fin[0] = None
                        pop_out(2)
                        emit_av(NP - 1)

                        def fin(h=h, acc_d=acc_d, ps_ctx=ps_ctx):
                            # fold den columns -> [128,512], colsum via
                            # ones-matmul (ps_s ring psum), reciprocal, mul
                            fold = den_p.tile([128, 512], f16, name="fold",
                                              tag="acc")
                            nc.vector.tensor_add(fold[:], acc_d[:, 0:512],
                                                 acc_d[:, 512:1024])
                            ps_den = ps_s_p.tile([128, 1024], f32,
                                                 name="psd", tag="pss")
                            nc.tensor.matmul(ps_den[:, 0:512], ones16[:],
                                             fold[:], start=True, stop=True)
                            rden = rden_p.tile([128, 512], f32, name="rden",
                                               tag="rden")
                            nc.vector.reciprocal_approx_fast(
                                rden[:], ps_den[:, 0:512])
                            c_t = ctx_p.tile([128, 512], f16, name="ctxt",
                                             tag="ctx")
                            nc.vector.tensor_mul(c_t[:], ps_ctx[:], rden[:])
                            ctx_sb[h] = c_t

                        pending_fin[0] = fin

                    pending_fin[0]()
                    pending_fin[0] = None
                    pop_out(64)  # drain any remaining qc-1 out-proj work
                    prev_out = make_out_emitters(qc, ctx_sb,
                                                 final=(qc == NQ - 1))

                # out-projection of the last q chunk
                for em in prev_out:
                    em()

    nc.compile()
    return nc


def _get_program():
    if "nc" not in _prog_cache:
        _prog_cache["nc"] = _build_program()
    return _prog_cache["nc"]


def _host_shards(x, W_comp, W_q_dec, W_k_dec, W_v_dec, W_rope_q, W_rope_k,
                 W_out):
    inv = 1.0 / (10000.0 ** (np.arange(0, RD, 2, dtype=np.float32) / RD))
    ang = np.arange(S, dtype=np.float32)[:, None] * inv[None, :]  # [S, 32]
    cosT = np.cos(ang).T.astype(np.float32)                       # [32, S]
    sinT = np.sin(ang).T.astype(np.float32)
    cos4 = np.ascontiguousarray(np.tile(cosT, (4, 1))).astype(np.float16)
    sin4s = np.ascontiguousarray(
        np.concatenate([-sinT, sinT], axis=0)).astype(np.float16)

    in_maps = []
    for c in range(NC):
        b, hg = divmod(c, 4)
        xTb = np.ascontiguousarray(x[b].T.astype(np.float16))
        w_big = np.ascontiguousarray(np.concatenate(
            [W_comp,
             W_rope_q[:, hg * 256:(hg + 1) * 256],
             W_rope_k[:, hg * 256:(hg + 1) * 256]],
            axis=1).astype(np.float16))
        w_qk = np.ascontiguousarray(np.concatenate(
            [W_q_dec[:, hg * 256:(hg + 1) * 256],
             W_k_dec[:, hg * 256:(hg + 1) * 256]],
            axis=1).astype(np.float16))
        w_v = np.ascontiguousarray(np.concatenate(
            [W_v_dec[:, hg * 256:(hg + 1) * 256],
             W_v_dec[:, 1024 + hg * 256:1024 + (hg + 1) * 256]],
            axis=1).astype(np.float16))
        w_o = np.ascontiguousarray(np.concatenate(
            [W_out[hg * 256:(hg + 1) * 256, :],
             W_out[1024 + hg * 256:1024 + (hg + 1) * 256, :]],
            axis=0).astype(np.float16))
        in_maps.append({
            "xT": xTb, "w_big": w_big, "w_qk": w_qk, "w_v": w_v, "w_o": w_o,
            "cos4": cos4, "sin4s": sin4s,
        })
    return in_maps


def kernel(x, W_comp, W_q_dec, W_k_dec, W_v_dec, W_rope_q, W_rope_k, W_out,
           _trace=False):
    from concourse import bass_utils

    x = np.asarray(x, np.float32)
    args = [np.asarray(a, np.float32)
            for a in (W_comp, W_q_dec, W_k_dec, W_v_dec,
                      W_rope_q, W_rope_k, W_out)]
    in_maps = _host_shards(x, *args)
    nc = _get_program()
    res = bass_utils.run_bass_kernel_spmd(
        nc, in_maps, core_ids=list(range(NC)), trace=_trace)
    out = np.zeros((B, S, D), np.float32)
    for c in range(NC):
        b = c // 4
        out[b] += res.results[c]["out"].T.astype(np.float32)
    if _trace:
        kernel.last_exec_ns = res.exec_time_ns
    return out


# revision 23
# speedup vs baseline: 1.0138x; 1.0138x over previous
"""Multi-Head Latent Attention (MLA) Trainium2 kernel, v6 (fp16).

Problem (hardcoded): B=2, S=2048, D_MODEL=2048, H=16, HEAD_DIM=128,
D_LATENT=512 (D_QK=256 / D_V=256), ROPE_DIM=64, fp32 in/out.

Reference semantics: q = concat([q_no_rope(1024), q_rope(1024)]).reshape(16
heads x 128), so heads 0-7 take both 64-dim halves from the latent
decompression and heads 8-15 take both halves from the rope projection of x;
RoPE rotates dims 64:128 of every head.

Sharding: 8 cores = 2 batches x 4 head-groups; core (b, hg) owns heads
[2hg, 2hg+1, 8+2hg, 8+2hg+1] (2 decompression + 2 rope-projection heads),
computes the shared latent for its batch redundantly, and produces a partial
output projection (its heads' rows of W_out), transposed [e, q]. The host
sums the 4 partials per batch.

v3 structure:
  - fp16 operands everywhere (host converts), f32 psum accumulation
  - stage1 n-outer: xT streamed in 512-column chunks so the PE starts after
    ~6MB of DMA instead of 13MB; 8 psum banks = 8 m-blocks per chunk
  - attention per (qc, h): scores in [128,1024] kc-pair psums, exp with
    bias=-3 shift on scalar, av accumulating in one psum bank
  - softmax denominator: exp-tile accumulation split between DVE and GpSimd
    (both SBUF-only), folded then column-summed with one small ones-matmul
  - out-projection matmuls of q-chunk qc-1 are interleaved two-per-pair-step
    into attention(qc) so the PE fills the exp-latency gaps
"""

import math

import numpy as np

B = 2
S = 2048
D = 2048
H4 = 4            # heads per core
HD = 128          # head dim
DL = 512          # d_latent
DQK = 256
RD = 64           # rope dim
NC = 8            # cores

NQ = S // 512     # 4 q chunks of 512
NK = S // 128     # 16 k chunks of 128
NP = NK // 2      # 8 kc pairs
KD = D // 128     # 16 contraction chunks for stage 1

SCALE = 1.0 / math.sqrt(HD)
EXP_SHIFT = -3.0  # exp(s*SCALE - 3): cancels in softmax, keeps fp16 range

_prog_cache = {}


def _build_program():
    import contextlib

    import concourse.tile as tile
    from concourse import bacc, mybir

    f16 = mybir.dt.float16
    f32 = mybir.dt.float32
    EXP = mybir.ActivationFunctionType.Exp
    IDENT = mybir.ActivationFunctionType.Identity

    nc = bacc.Bacc("TRN2", target_bir_lowering=False, debug=False,
                   num_devices=1)

    xT = nc.dram_tensor("xT", [D, S], f16, kind="ExternalInput")
    w_big = nc.dram_tensor("w_big", [D, 1024], f16, kind="ExternalInput")
    w_qk = nc.dram_tensor("w_qk", [DQK, 512], f16, kind="ExternalInput")
    w_v = nc.dram_tensor("w_v", [DQK, 512], f16, kind="ExternalInput")
    w_o = nc.dram_tensor("w_o", [DL, D], f16, kind="ExternalInput")
    cos4_d = nc.dram_tensor("cos4", [128, S], f16, kind="ExternalInput")
    sin4s_d = nc.dram_tensor("sin4s", [64, S], f16, kind="ExternalInput")
    out_d = nc.dram_tensor("out", [D, S], f16, kind="ExternalOutput")

    with tile.TileContext(nc, pool_alloc_mode="queue") as tc:
        with contextlib.ExitStack() as ctx:
            # ---------------- persistent pools ----------------
            ones_p = ctx.enter_context(tc.tile_pool(name="onesp", bufs=1))
            qk_p = ctx.enter_context(tc.tile_pool(name="qk", bufs=1))
            v_p = ctx.enter_context(tc.tile_pool(name="vp", bufs=1))
            consts_p = ctx.enter_context(tc.tile_pool(name="consts", bufs=1))
            wdec_p = ctx.enter_context(tc.tile_pool(name="wdec", bufs=1))

            ones_f32 = ones_p.tile([128, 128], f32)
            nc.gpsimd.memset(ones_f32[:], 1.0)
            ones16 = ones_p.tile([128, 128], f16)
            nc.vector.tensor_copy(ones16[:], ones_f32[:])
            warm = ones_p.tile([128, 1], f32)
            nc.scalar.activation(warm[:], ones_f32[:, 0:1], EXP)
            shift = ones_p.tile([128, 1], f32)
            nc.gpsimd.memset(shift[:], EXP_SHIFT)

            # qkT[0..3] = q heads 0..3, qkT[4..7] = k heads 0..3
            # (local heads 0,1 = dec heads; 2,3 = rope-proj heads)
            qkT = [qk_p.tile([128, S], f16, name=f"qkT{i}", tag=f"qk{i}")
                   for i in range(8)]
            # v pair tiles: v_pair[p] = v_nat[2p] | v_nat[2p+1]
            v_pair = [v_p.tile([128, 1024], f16, name=f"v{p}", tag=f"v{p}")
                      for p in range(NP)]

            # ---------------- phase A: stage1 + dec + v + rope ----------
            # latn[i] = latent rows [128i,128i+128) x [S], fp16
            # (i=0,1: c_qk; i=2,3: c_v)
            lat_cm = tc.tile_pool(name="lat", bufs=1)
            lat_p = lat_cm.__enter__()
            latn = [lat_p.tile([128, S], f16, name=f"latT{i}", tag=f"lat{i}")
                    for i in range(4)]

            wbig_cm = tc.tile_pool(name="wbig", bufs=1)
            wbig_p = wbig_cm.__enter__()
            # xt ring: 32 x [128,512] column-chunk tiles (2 n-chunks in
            # flight); DMA of chunk n+2 waits for chunk n's slots to free
            xt_cm = tc.tile_pool(name="xt", bufs=32)
            xt_p = xt_cm.__enter__()

            wbig_sb = []
            xt_sb = [[None] * KD for _ in range(4)]
            for k in range(KD):
                w_t = wbig_p.tile([128, 1024], f16, name=f"wb{k}")
                nc.sync.dma_start(
                    w_t[:], w_big.ap()[k * 128:(k + 1) * 128, :])
                wbig_sb.append(w_t)
                x_t = xt_p.tile([128, 512], f16, name=f"xt0_{k}", tag="xt")
                nc.sync.dma_start(x_t[:],
                                  xT.ap()[k * 128:(k + 1) * 128, 0:512])
                xt_sb[0][k] = x_t
            for n in range(1, 4):
                for k in range(KD):
                    x_t = xt_p.tile([128, 512], f16, name=f"xt{n}_{k}",
                                    tag="xt")
                    nc.sync.dma_start(
                        x_t[:],
                        xT.ap()[k * 128:(k + 1) * 128,
                                n * 512:(n + 1) * 512])
                    xt_sb[n][k] = x_t

            cos4 = consts_p.tile([128, S], f16)
            nc.sync.dma_start(cos4[:], cos4_d.ap()[:])
            sin4s = consts_p.tile([64, S], f16)
            nc.sync.dma_start(sin4s[:], sin4s_d.ap()[:])

            wqk_sb = []
            for l in range(2):
                w_t = wdec_p.tile([128, 512], f16, name=f"wqk{l}")
                nc.sync.dma_start(w_t[:], w_qk.ap()[l * 128:(l + 1) * 128, :])
                wqk_sb.append(w_t)
            wv_sb = []
            for l in range(2):
                w_t = wdec_p.tile([128, 512], f16, name=f"wv{l}")
                nc.sync.dma_start(w_t[:], w_v.ap()[l * 128:(l + 1) * 128, :])
                wv_sb.append(w_t)

            psA_cm = tc.tile_pool(name="psA", bufs=8, space="PSUM")
            psA_p = psA_cm.__enter__()

            sw_cm = tc.tile_pool(name="swp", bufs=2)
            sw_p = sw_cm.__enter__()
            scr_cm = tc.tile_pool(name="scr", bufs=2)
            scr_p = scr_cm.__enter__()

            def rope_tile(i):
                # in-place RoPE on rows 64:128 of qkT[i]
                t = qkT[i]
                sw = sw_p.tile([64, S], f16, name=f"sw{i}", tag="sw")
                nc.sync.dma_start(sw[0:32, :], t[96:128, :])
                nc.sync.dma_start(sw[32:64, :], t[64:96, :])
                tmp_sin = scr_p.tile([64, S], f16, name="tsin", tag="scr")
                nc.vector.tensor_mul(tmp_sin[:], sw[:], sin4s[:])
                tmp_cos = scr_p.tile([64, S], f16, name="tcos", tag="scr")
                nc.vector.tensor_mul(tmp_cos[:], t[64:128, :],
                                     cos4[64:128, :])
                nc.vector.tensor_add(t[64:128, :], tmp_cos[:], tmp_sin[:])

            # stage1: n-outer, all 8 m-blocks per 512-column chunk
            s1_dst = [latn[0], latn[1], latn[2], latn[3],
                      qkT[2], qkT[3], qkT[6], qkT[7]]
            for n in range(4):
                nsl = slice(n * 512, (n + 1) * 512)
                M_ORD = (4, 6, 5, 7, 0, 1, 2, 3)  # rope-head blocks first
                pss = {m: psA_p.tile([128, 512], f32, name=f"ps_{n}_{m}",
                                     tag="psA") for m in M_ORD}
                for k in range(KD):
                    for m in M_ORD:
                        nc.tensor.matmul(
                            pss[m][:],
                            wbig_sb[k][:, m * 128:(m + 1) * 128],
                            xt_sb[n][k][:],
                            start=(k == 0), stop=(k == KD - 1),
                        )
                for m in M_ORD:
                    nc.scalar.activation(s1_dst[m][:, nsl], pss[m][:], IDENT)

            # rope-proj heads rotate first (feed attention heads 2,3)
            rope_tile(2)
            rope_tile(6)

            def dec_mt(mt, on_dve=False):
                # q/k decompression for dec head mt -> qkT[[0,1,4,5][mt]]
                dst = qkT[[0, 1, 4, 5][mt]]
                pss = [psA_p.tile([128, 512], f32, name=f"psd{mt}_{n2}",
                                  tag="psA") for n2 in range(4)]
                for l in range(2):
                    for n2 in range(4):
                        nc.tensor.matmul(
                            pss[n2][:],
                            wqk_sb[l][:, mt * 128:(mt + 1) * 128],
                            latn[l][:, n2 * 512:(n2 + 1) * 512],
                            start=(l == 0), stop=(l == 1),
                        )
                for n2 in range(4):
                    if on_dve:
                        nc.vector.tensor_copy(
                            dst[:, n2 * 512:(n2 + 1) * 512], pss[n2][:])
                    else:
                        nc.scalar.activation(
                            dst[:, n2 * 512:(n2 + 1) * 512], pss[n2][:],
                            IDENT)

            # v decompression first (feeds av of heads 2,3; casts on DVE
            # so the scalar queue reaches the first exps quickly)
            for sc in range(NK):
                ps = psA_p.tile([128, 512], f32, name=f"psv{sc}", tag="psA")
                for l in range(2):
                    nc.tensor.matmul(
                        ps[:],
                        latn[2 + l][:, sc * 128:(sc + 1) * 128],
                        wv_sb[l][:],
                        start=(l == 0), stop=(l == 1),
                    )
                nc.scalar.activation(
                    v_pair[sc // 2][:, (sc % 2) * 512:(sc % 2 + 1) * 512],
                    ps[:], IDENT)
            dec_mt(0)
            dec_mt(2)
            dec_mt(1)
            dec_mt(3)
            rope_tile(3)
            rope_tile(7)
            rope_tile(0)
            rope_tile(4)
            rope_tile(1)
            rope_tile(5)

            scr_cm.__exit__(None, None, None)
            sw_cm.__exit__(None, None, None)
            psA_cm.__exit__(None, None, None)
            xt_cm.__exit__(None, None, None)
            wbig_cm.__exit__(None, None, None)
            lat_cm.__exit__(None, None, None)

            # ---------------- phase B: attention + out projection --------
            # PSUM: ps_s 2x[128,1024] (4 banks) + ps_c 2x[128,512] (2) +
            #       ps_o 1x[128,1024] (2) = 8 banks
            with tc.tile_pool(name="wo", bufs=1) as wo_p, \
                 tc.tile_pool(name="exp", bufs=3) as exp_p, \
                 tc.tile_pool(name="den", bufs=4) as den_p, \
                 tc.tile_pool(name="rden", bufs=2) as rden_p, \
                 tc.tile_pool(name="ctx", bufs=8) as ctx_p, \
                 tc.tile_pool(name="stage", bufs=3) as stage_p, \
                 tc.tile_pool(name="ps_s", bufs=2, space="PSUM") as ps_s_p, \
                 tc.tile_pool(name="ps_c", bufs=2, space="PSUM") as ps_c_p, \
                 tc.tile_pool(name="ps_o", bufs=1, space="PSUM") as ps_o_p:
                wo_sb = []
                for kk in range(4):
                    w_t = wo_p.tile([128, D], f16, name=f"wo{kk}")
                    nc.sync.dma_start(
                        w_t[:], w_o.ap()[kk * 128:(kk + 1) * 128, :])
                    wo_sb.append(w_t)

                def make_out_emitters(qc, ctx_tiles, final=False):
                    """64 single-matmul emitters for out-proj of q-chunk qc,
                    with psum alloc / cast / dma folded into the stream."""
                    qsl = slice(qc * 512, (qc + 1) * 512)
                    state = {}

                    def emit_one(i):
                        mp, r = divmod(i, 8)
                        half, kk = divmod(r, 4)
                        m = 2 * mp + half
                        if r == 0:
                            use_s = final and (mp % 3 != 0)
                            pool = ps_s_p if use_s else ps_o_p
                            tag2 = "pss" if use_s else "pso"
                            state["ps"] = pool.tile(
                                [128, 1024], f32, name="pso", tag=tag2)
                        nc.tensor.matmul(
                            state["ps"][:, half * 512:(half + 1) * 512],
                            wo_sb[kk][:, m * 128:(m + 1) * 128],
                            ctx_tiles[kk][:],
                            start=(kk == 0), stop=(kk == 3),
                        )
                        if r in (3, 7):
                            hf = r // 4
                            st = stage_p.tile([128, 512], f16, name="stg",
                                              tag="stage")
                            psl = state["ps"][:, hf * 512:(hf + 1) * 512]
                            if hf == 0:
                                nc.vector.tensor_copy(st[:], psl)
                            else:
                                nc.scalar.activation(st[:], psl, IDENT)
                            nc.sync.dma_start(
                                out_d.ap()[m * 128:(m + 1) * 128, qsl],
                                st[:])

                    return [lambda i=i: emit_one(i) for i in range(64)]

                prev_out = []
                pending_fin = [None]
                for qc in range(NQ):
                    qsl = slice(qc * 512, (qc + 1) * 512)
                    out_q = list(prev_out)  # out-proj work of qc-1
                    oi = 0

                    def pop_out(k2, out_q=out_q):
                        nonlocal oi
                        for _ in range(k2):
                            if oi < len(out_q):
                                out_q[oi]()
                                oi += 1

                    ctx_sb = [None] * H4
                    for h in (2, 3, 0, 1):
                        ps_ctx = ps_c_p.tile([128, 512], f32, name="psc",
                                             tag="psc")
                        acc_d = den_p.tile([128, 1024], f16, name="accd",
                                           tag="acc")
                        exps = []

                        def emit_scores(p, h=h, qsl=qsl, exps=exps,
                                        qc=qc):
                            # qc0 runs a 2-ahead schedule; borrow the idle
                            # ps_o pool every 3rd pair for a 3-deep ring
                            deep = qc == 0 and p % 3 == 2
                            pool = ps_o_p if deep else ps_s_p
                            ps = pool.tile([128, 1024], f32, name="pss",
                                           tag="pso" if deep else "pss")
                            for j in range(2):
                                kc = 2 * p + j
                                nc.tensor.matmul(
                                    ps[:, j * 512:(j + 1) * 512],
                                    qkT[4 + h][:, kc * 128:(kc + 1) * 128],
                                    qkT[h][:, qsl],
                                    start=True, stop=True,
                                )
                            e = exp_p.tile([128, 1024], f16, name="expT",
                                           tag="exp")
                            nc.scalar.activation(e[:], ps[:], EXP,
                                                 bias=shift[:], scale=SCALE)
                            exps.append(e)

                        def emit_av(p, h=h, exps=exps, ps_ctx=ps_ctx,
                                    acc_d=acc_d):
                            e = exps[p]
                            for j in range(2):
                                kc = 2 * p + j
                                nc.tensor.matmul(
                                    ps_ctx[:],
                                    v_pair[p][:, j * 512 + h * 128:
                                              j * 512 + (h + 1) * 128],
                                    e[:, j * 512:(j + 1) * 512],
                                    start=(kc == 0), stop=(kc == NK - 1),
                                )
                            # denominator accumulation on DVE (fp16 2x)
                            if p == 0:
                                nc.vector.tensor_copy(acc_d[:], e[:])
                            else:
                                nc.vector.tensor_add(acc_d[:], acc_d[:],
                                                     e[:])

                        if qc == 0:
                            # no out-proj work to interleave: cover exp
                            # latency with a 2-ahead scores schedule
                            emit_scores(0)
                            emit_scores(1)
                            emit_scores(2)
                            emit_av(0)
                            for p in range(3, NP):
                                emit_scores(p)
                                emit_av(p - 2)
                                if p == 4 and pending_fin[0] is not None:
                                    pending_fin[0]()
                                    pending_fin[0] = None
                            emit_av(NP - 2)
                            emit_av(NP - 1)
                        else:
                            emit_scores(0)
                            emit_scores(1)
                            pop_out(2)
                            emit_av(0)
                            for p in range(2, NP):
                                emit_scores(p)
                                pop_out(2)
                                emit_av(p - 1)
                                if p == 3 and pending_fin[0] is not None:
                                    # previous head's den/ctx finisher,
                                    # late enough that its fold is ready
                                    pending_fin[0]()
                                    pending_fin[0] = None
                            pop_out(2)
                            emit_av(NP - 1)

                        def fin(h=h, acc_d=acc_d, ps_ctx=ps_ctx):
                            # fold den columns -> [128,512], colsum via
                            # ones-matmul (ps_s ring psum), reciprocal, mul
                            fold = den_p.tile([128, 512], f16, name="fold",
                                              tag="acc")
                            nc.vector.tensor_add(fold[:], acc_d[:, 0:512],
                                                 acc_d[:, 512:1024])
                            ps_den = ps_s_p.tile([128, 1024], f32,
                                                 name="psd", tag="pss")
                            nc.tensor.matmul(ps_den[:, 0:512], ones16[:],
                                             fold[:], start=True, stop=True)
                            rden = rden_p.tile([128, 512], f32, name="rden",
                                               tag="rden")
                            nc.vector.reciprocal_approx_fast(
                                rden[:], ps_den[:, 0:512])
                            c_t = ctx_p.tile([128, 512], f16, name="ctxt",
                                             tag="ctx")
                            nc.vector.tensor_mul(c_t[:], ps_ctx[:], rden[:])
                            ctx_sb[h] = c_t

                        pending_fin[0] = fin

                    pending_fin[0]()
                    pending_fin[0] = None
                    pop_out(64)  # drain any remaining qc-1 out-proj work
                    prev_out = make_out_emitters(qc, ctx_sb,
                                                 final=(qc == NQ - 1))

                # out-projection of the last q chunk
                for em in prev_out:
                    em()

    nc.compile()
    return nc


def _get_program():
    if "nc" not in _prog_cache:
        _prog_cache["nc"] = _build_program()
    return _prog_cache["nc"]


def _host_shards(x, W_comp, W_q_dec, W_k_dec, W_v_dec, W_rope_q, W_rope_k,
                 W_out):
    inv = 1.0 / (10000.0 ** (np.arange(0, RD, 2, dtype=np.float32) / RD))
    ang = np.arange(S, dtype=np.float32)[:, None] * inv[None, :]  # [S, 32]
    cosT = np.cos(ang).T.astype(np.float32)                       # [32, S]
    sinT = np.sin(ang).T.astype(np.float32)
    cos4 = np.ascontiguousarray(np.tile(cosT, (4, 1))).astype(np.float16)
    sin4s = np.ascontiguousarray(
        np.concatenate([-sinT, sinT], axis=0)).astype(np.float16)

    in_maps = []
    for c in range(NC):
        b, hg = divmod(c, 4)
        xTb = np.ascontiguousarray(x[b].T.astype(np.float16))
        w_big = np.ascontiguousarray(np.concatenate(
            [W_comp,
             W_rope_q[:, hg * 256:(hg + 1) * 256],
             W_rope_k[:, hg * 256:(hg + 1) * 256]],
            axis=1).astype(np.float16))
        w_qk = np.ascontiguousarray(np.concatenate(
            [W_q_dec[:, hg * 256:(hg + 1) * 256],
             W_k_dec[:, hg * 256:(hg + 1) * 256]],
            axis=1).astype(np.float16))
        w_v = np.ascontiguousarray(np.concatenate(
            [W_v_dec[:, hg * 256:(hg + 1) * 256],
             W_v_dec[:, 1024 + hg * 256:1024 + (hg + 1) * 256]],
            axis=1).astype(np.float16))
        w_o = np.ascontiguousarray(np.concatenate(
            [W_out[hg * 256:(hg + 1) * 256, :],
             W_out[1024 + hg * 256:1024 + (hg + 1) * 256, :]],
            axis=0).astype(np.float16))
        in_maps.append({
            "xT": xTb, "w_big": w_big, "w_qk": w_qk, "w_v": w_v, "w_o": w_o,
            "cos4": cos4, "sin4s": sin4s,
        })
    return in_maps


def kernel(x, W_comp, W_q_dec, W_k_dec, W_v_dec, W_rope_q, W_rope_k, W_out,
           _trace=False):
    from concourse import bass_utils

    x = np.asarray(x, np.float32)
    args = [np.asarray(a, np.float32)
            for a in (W_comp, W_q_dec, W_k_dec, W_v_dec,
                      W_rope_q, W_rope_k, W_out)]
    in_maps = _host_shards(x, *args)
    nc = _get_program()
    res = bass_utils.run_bass_kernel_spmd(
        nc, in_maps, core_ids=list(range(NC)), trace=_trace)
    out = np.zeros((B, S, D), np.float32)
    for c in range(NC):
        b = c // 4
        out[b] += res.results[c]["out"].T.astype(np.float32)
    if _trace:
        kernel.last_exec_ns = res.exec_time_ns
    return out
